# revision 56
# baseline (speedup 1.0000x reference)
"""Trainium2 Bass kernel for nn_MultiHeadAttention (B=2, S=2048, D=1024, H=16).

Sharding: 8 cores = 2 (batch) x 4 (head groups of 4 heads / 256 dims).
Each core computes QKV projections for its head slice, attention for its 4
heads, and the partial output projection for its 256-dim slice of Wo's input.
Host sums the 4 partials per batch element (Megatron-style row-parallel Wo).

Device layouts (per core):
  qT/kT/vT  [1024, 2048] bf16   (input, transposed on host)
  wqT/wkT/wvT [1024, 256] bf16  (Wq[js].T etc)
  woT       [256, 1024] bf16    (Wo[:, js].T)
  maskT     [2048, 2048] bf16   (mask[0,0].T as 0.0/1.0)
  qpT/kpT   [256(j), 2048(s)]   (projections, transposed: j on partitions)
  vp        [2048(t), 4x65]     (natural layout; col 64 of each 65-block = 1.0
                                 -> attn@V matmul also produces softmax denom)
  P~        [t, s] = exp(scoresT/8) * maskT   (scoresT = K_h.T^T @ Q_h.T)
  attn out  [65(j+denom), s] -> normalized -> concatT [256(j), 2048(s)]
  out_p     [2048, 1024] f32 partial = concatT.T @ woT
"""

import sys

import numpy as np

try:
    import concourse.bass as bass
except ImportError:  # pragma: no cover
    sys.path.insert(0, "/opt/trn_rl_repo")
    import concourse.bass as bass

from concourse import bacc

import ml_dtypes

import concourse.tile as tile_mod
from concourse import mybir
from concourse.bass_utils import run_bass_kernel_spmd

BF16 = ml_dtypes.bfloat16
F32 = np.float32

B, S, D, H = 2, 2048, 1024, 16
DK = D // H            # 64
N_CORES = 8
HPC = 4                # heads per core
JC = HPC * DK          # 256 j-dims per core
SCALE = 1.0 / float(np.sqrt(DK))
NSB = S // 512         # 4 s-blocks
NC_T = S // 128        # 16 t-chunks
VROW = HPC * 65        # 260: [h0 64 | 1 | h1 64 | 1 | ...]

bf = mybir.dt.bfloat16
f32 = mybir.dt.float32
f8 = mybir.dt.bfloat16   # fp8 inputs tried and reverted: logit noise 4.5e-2
F8 = BF16
WPRE = 1.0
WINV = 1.0 / WPRE


def _patch_drain():
    """This walrus build only accepts 1 sync-wait per instruction; the Tile
    exit drain carries one wait per pending proc. Split them across drains."""
    if getattr(tile_mod.TileContext, "_drain_patched", False):
        return
    import bass_rust

    def _drain_and_barrier(self, tick_clock, wait_clock):
        from concourse.tile import ScopedClock

        nc = self.nc
        drain_inst = nc.sync.drain()
        wait_clock.add_sem_waits(
            drain_inst.ins, ScopedClock({None: tick_clock.global_clock})
        )
        si = drain_inst.ins.sync_info
        waits = list(si.on_wait)
        if len(waits) > 1:
            drain_inst.ins.sync_info = bass_rust.SyncInfo(
                on_wait=[waits[0]], on_update=list(si.on_update)
            )
            for w in waits[1:]:
                d2 = nc.sync.drain()
                d2.ins.sync_info = bass_rust.SyncInfo(on_wait=[w], on_update=[])
        nc.all_engine_barrier()
        assert self.sems is not None
        popped = nc._tile_sem_poison_stack.pop()
        assert popped is self._sem_poison
        nc.clear_and_free_semaphores(list(self.sems.allocated().values()))
        nc.all_engine_barrier()

    tile_mod.TileContext._drain_and_barrier = _drain_and_barrier
    tile_mod.TileContext._drain_patched = True


SCH_A = SCALE * float(np.log2(np.e)) * 128.0   # exp2 bit-hack scale
SCH_B = 16256.0 - 7.42                          # 127*128 - sigma
DVE_CHUNKS = (8,)                               # chunks exp'd off the ACT engine


def _emit(tc, T):
    nc = tc.nc
    Exp = mybir.ActivationFunctionType.Exp
    i16 = mybir.dt.int16

    from contextlib import ExitStack

    with ExitStack() as ctx:
        persist = ctx.enter_context(tc.tile_pool(name="persist", bufs=1))

        # ---- weights / persistent tiles ----
        # DMA issue order matters: the DMA engines round-robin across active
        # queues, so anything issued early competes with the k/q blocks the
        # head needs first.  Sync queue: wk, wq, biasqk, kT0, qT0, kT1...
        # The wv/wo/mask/v issues are gated behind kproj(0) completion via a
        # dummy gpsimd read of kpT (emitted later).
        wq = persist.tile([128, 8 * JC], f8, tag="wq")
        wk = persist.tile([128, 8 * JC], f8, tag="wk")
        wv = persist.tile([128, 8 * JC], f8, tag="wv")
        for t, name in ((wk, "wkT"), (wq, "wqT")):
            nc.sync.dma_start(
                t[:].rearrange("p (c j) -> p c j", c=8),
                T[name][:, :].rearrange("(c p) j -> p c j", p=128),
            )
        biasqk = persist.tile([128, 4], f32, tag="biasqk")
        nc.sync.dma_start(biasqk[:], T["biasqk"][:, :])
        ones1 = persist.tile([1, 512], f32, tag="ones1")
        nc.gpsimd.memset(ones1[:], 1.0)
        wo = [persist.tile([128, D], bf, tag=f"wo{i}", name=f"wo{i}") for i in range(2)]

        # per-sb q/k projection tiles ([j, s] transposed layout)
        qpS = [
            [persist.tile([128, 512], bf, tag=f"qp{j}_{s}", name=f"qp{j}_{s}")
             for s in range(NSB)]
            for j in range(2)
        ]
        kpT = [
            [persist.tile([128, 1024], bf, tag=f"kpT{i}_{th}", name=f"kpT{i}_{th}")
             for th in range(2)]
            for i in range(2)
        ]
        # per-chunk v tiles (natural [t, j] layout + ones cols)
        vpc = [persist.tile([128, VROW], bf, tag=f"vp{c}", name=f"vp{c}")
               for c in range(NC_T)]
        concatT = [persist.tile([128, S], bf, tag=f"concatT{i}", name=f"concatT{i}") for i in range(2)]

        wq_v = wq[:].rearrange("p (c j) -> p c j", c=8)
        wk_v = wk[:].rearrange("p (c j) -> p c j", c=8)
        wv_v = wv[:].rearrange("p (c j) -> p c j", c=8)

        q_stream = ctx.enter_context(tc.tile_pool(name="q_stream", bufs=1))
        qtts = {}

        def emit_qdma(sb):
            sl = slice(sb * 512, (sb + 1) * 512)
            qTt = q_stream.tile([128, 8 * 512], f8, tag="qTt", name=f"qTt{sb}")
            nc.sync.dma_start(
                qTt[:].rearrange("p (c s) -> p c s", c=8),
                T["qT"][:, sl].rearrange("(c p) s -> p c s", p=128),
            )
            qtts[sb] = qTt[:].rearrange("p (c s) -> p c s", c=8)

        def emit_qproj_jt(sb, jt):
            jsl = slice(jt * 128, (jt + 1) * 128)
            ps = bigp.tile([128, 512], f32, tag="big", name=f"pq{sb}_{jt}")
            for c in range(8):
                nc.tensor.matmul(
                    ps[:], wq_v[:, c, jsl], qtts[sb][:, c, :],
                    start=(c == 0), stop=(c == 7),
                )
            nc.vector.tensor_scalar(
                qpS[jt][sb][:], ps[:], WINV, biasqk[:, jt : jt + 1],
                mybir.AluOpType.mult, mybir.AluOpType.add,
            )

        def emit_qproj(sb):
            emit_qdma(sb)
            emit_qproj_jt(sb, 0)
            emit_qproj_jt(sb, 1)

        # ---- attention + output projection ----
        # Chunk-level software pipeline: per t-chunk the PE stream carries
        # scores(i) for both heads (concurrent row-groups), then attnV(i-1)
        # for both heads, plus occasional "extras" (Wo / q-proj / v-proj /
        # mask prefetch). ACT (exp) is the pacing engine; this keeps it fed
        # every chunk while the PE stays dense enough to hold HAM at 8/8.
        if True:
            vstream = ctx.enter_context(tc.tile_pool(name="vstream", bufs=2))
            schp = ctx.enter_context(tc.tile_pool(name="schp", bufs=2))
            maskp = ctx.enter_context(tc.tile_pool(name="maskp", bufs=2))
            ptp = ctx.enter_context(tc.tile_pool(name="ptp", bufs=2))
            smallp = ctx.enter_context(tc.tile_pool(name="smallp", bufs=1))
            outp = ctx.enter_context(tc.tile_pool(name="outp", bufs=2))
            scp = ctx.enter_context(tc.tile_pool(name="scp", bufs=2, space="PSUM"))
            mtiles = {}

        # ---- k/q projections pipelined into the attention loop ----
        bigp = ctx.enter_context(tc.tile_pool(name="bigp", bufs=4, space="PSUM"))
        with tc.tile_pool(name="kv_stream", bufs=2) as kv_stream:
            ktts = {}

            def emit_kdma(sb):
                sl = slice(sb * 512, (sb + 1) * 512)
                kTt = kv_stream.tile([128, 8 * 512], f8, tag="kTt", name=f"kTt{sb}")
                nc.sync.dma_start(
                    kTt[:].rearrange("p (c s) -> p c s", c=8),
                    T["kT"][:, sl].rearrange("(c p) s -> p c s", p=128),
                )
                ktts[sb] = kTt[:].rearrange("p (c s) -> p c s", c=8)

            def emit_kproj_jt(sb, jt):
                jsl = slice(jt * 128, (jt + 1) * 128)
                ps = bigp.tile([128, 512], f32, tag="big", name=f"pk{sb}_{jt}")
                for c in range(8):
                    nc.tensor.matmul(
                        ps[:], wk_v[:, c, jsl], ktts[sb][:, c, :],
                        start=(c == 0), stop=(c == 7),
                    )
                nc.vector.tensor_scalar(
                    kpT[jt][sb // 2][:, (sb % 2) * 512 : (sb % 2 + 1) * 512],
                    ps[:], WINV, biasqk[:, 2 + jt : 3 + jt],
                    mybir.AluOpType.mult, mybir.AluOpType.add,
                )

            # Head: get k0/q0 in flight first, project them, then start
            # attention immediately; k1-k3 projections run as extras inside
            # the (0,0) chunk loop (the PE is otherwise idle there since
            # there is no previous attnV to overlap).
            emit_kdma(0)
            emit_qdma(0)
            emit_kdma(1)
            emit_kproj_jt(0, 0)
            emit_kproj_jt(0, 1)
            emit_qproj_jt(0, 0)
            emit_qproj_jt(0, 1)
            emit_kdma(2)
            emit_kdma(3)
            # Gate the second DMA wave (wv/v/mask/wo) behind kproj(0) so the
            # round-robin DMA engines give the head's k/q blocks full
            # bandwidth first.  Tile schedules by data dependency (not
            # program order), so each gated DMA needs a REAL dep: write a
            # corner of its destination tile from kpT first (WAW ordering).
            def gate(dst_corner):
                nc.gpsimd.tensor_copy(dst_corner, kpT[0][0][0:1, 0:2])

            gate(wv[0:1, 0:2])
            nc.gpsimd.dma_start(
                wv[:].rearrange("p (c j) -> p c j", c=8),
                T["wvT"][:, :].rearrange("(c p) j -> p c j", p=128),
            )




            def emit_mask_dma(sb, half, gated=False):
                # split per t-half so the piece needed by the c==7 mask-mul
                # lands first
                sl = slice(sb * 512, (sb + 1) * 512)
                if half == 0:
                    mT = maskp.tile(
                        [128, NC_T * 512], bf, tag="mT", name=f"mT{sb}"
                    )
                    mtiles[sb] = mT
                mT = mtiles[sb]
                if gated:
                    gate(mT[0:1, half * 4096 : half * 4096 + 2])
                csl = slice(half * 8, half * 8 + 8)
                nc.gpsimd.dma_start(
                    mT[:].rearrange("p (c s) -> p c s", c=NC_T)[:, csl, :],
                    T["maskT"][half * 1024 : (half + 1) * 1024, sl].rearrange(
                        "(c p) s -> p c s", p=128
                    ),
                )

            vtts = {}

            def emit_vdma(tb, gated=False):
                sl = slice(tb * 512, (tb + 1) * 512)
                vTt = vstream.tile([128, 8 * 512], f8, tag="vTt", name=f"vTt{tb}")
                if gated:
                    gate(vTt[0:1, 0:2])
                nc.gpsimd.dma_start(
                    vTt[:].rearrange("p (c s) -> p c s", c=8),
                    T["vT"][:, sl].rearrange("(c p) s -> p c s", p=128),
                )
                vtts[tb] = vTt[:].rearrange("p (c t) -> p c t", c=8)

            def emit_vproj_tb(tb):
                vTt_v = vtts[tb]
                if tb + 2 < NSB and tb + 2 not in vtts:
                    emit_vdma(tb + 2)
                for tt in range(4):
                    chunk = tb * 4 + tt
                    ps = bigp.tile([128, 512], f32, tag="big", name=f"pv{chunk}")
                    for c in range(8):
                        nc.tensor.matmul(
                            ps[:, 0:JC],
                            vTt_v[:, c, tt * 128 : (tt + 1) * 128],
                            wv_v[:, c, :],
                            start=(c == 0), stop=(c == 7),
                        )
                    vt = vpc[chunk]
                    nc.gpsimd.memset(
                        vt[:].rearrange("p (h d) -> p h d", d=65)[:, :, 64:65],
                        1.0,
                    )
                    dst = vt[:].rearrange("p (h d) -> p h d", h=HPC)[:, :, 0:DK]
                    src = ps[:, 0:JC].rearrange("p (h d) -> p h d", h=HPC)
                    nc.vector.tensor_scalar_mul(dst, src, WINV)

            def emit_wo_group(sb, st, mt):
                s0 = sb * 512 + st * 128
                msl = slice(mt * 512, (mt + 1) * 512)
                pw = bigp.tile([128, 512], f32, tag="big", name=f"pw{sb}_{st}_{mt}")
                for kc in range(2):
                    nc.tensor.matmul(
                        pw[:],
                        concatT[kc][:, s0 : s0 + 128],
                        wo[kc][:, msl],
                        start=(kc == 0), stop=(kc == 1),
                    )
                ot = outp.tile([128, 512], bf, tag="ot", name=f"ot{sb}_{st}_{mt}")
                nc.vector.tensor_copy(ot[:], pw[:])
                nc.sync.dma_start(T["out_p"][s0 : s0 + 128, msl], ot[:])

            def emit_norm(sb, pair, po2):
                sl = slice(sb * 512, (sb + 1) * 512)
                rc0 = smallp.tile([1, 1024], f32, tag="rc0", name=f"rc0_{sb}_{pair}")
                for h2 in range(2):
                    nc.vector.tensor_copy(
                        rc0[0:1, h2 * 512 : (h2 + 1) * 512], po2[h2][64:65, :]
                    )
                rc = smallp.tile([1, 1024], f32, tag="rc", name=f"rc{sb}_{pair}")
                nc.vector.reciprocal_approx_fast(rc[:], rc0[:])
                for h2 in range(2):
                    h = pair * 2 + h2
                    psl = slice(h2 * 64, h2 * 64 + 64)
                    rb = smallp.tile([64, 512], f32, tag="rb", name=f"rb{sb}_{h}")
                    nc.gpsimd.partition_broadcast(
                        rb[:], rc[0:1, h2 * 512 : (h2 + 1) * 512], channels=64
                    )
                    nc.vector.tensor_mul(
                        concatT[pair][psl, sl], po2[h2][0:64, :], rb[:]
                    )

            emit_mask_dma(0, 0, gated=True)
            emit_vdma(0, gated=True)
            emit_vdma(1, gated=True)
            emit_mask_dma(0, 1, gated=True)
            for i in range(2):
                gate(wo[i][0:1, 0:2])
                nc.gpsimd.dma_start(wo[i][:], T["woT"][i * 128 : (i + 1) * 128, :])
            extras = [
                (lambda s=sbn, j=jt: emit_kproj_jt(s, j))
                for sbn in (1, 2, 3)
                for jt in range(2)
            ]
            po2L = None
            prev = None        # (sb, pair, Pt, po2)
            for sb in range(NSB):
                for pair in range(2):
                    last_it = (sb == NSB - 1 and pair == 1)
                    if sb == 0:
                        extras.append(lambda t=2 * pair: emit_vproj_tb(t))
                        extras.append(lambda t=2 * pair + 1: emit_vproj_tb(t))

                    Pt = ptp.tile(
                        [128, 2 * NC_T * 512], bf, tag="Pt", name=f"Pt{sb}_{pair}"
                    )
                    pv = Pt[:].rearrange("p (c h s) -> p c h s", c=NC_T, h=2)
                    mv = mtiles[sb][:].rearrange("p (c s) -> p c s", c=NC_T)
                    if prev is not None:
                        po2 = [
                            bigp.tile([128, 512], f32, tag="big",
                                      name=f"av{prev[0]}_{prev[1]}_{h2}")
                            for h2 in range(2)
                        ]
                    for c in range(NC_T):
                        ps = scp.tile(
                            [128, 1024], f32, tag="sc", name=f"sc{sb}_{pair}_{c}"
                        )
                        for h2 in range(2):
                            psl = slice(h2 * 64, h2 * 64 + 64)
                            nc.tensor.matmul(
                                ps[:, h2 * 512 : (h2 + 1) * 512],
                                kpT[pair][c // 8][psl, (c % 8) * 128 : (c % 8 + 1) * 128],
                                qpS[pair][sb][psl, :],
                                start=True, stop=True,
                            )
                        if c in DVE_CHUNKS:
                            # exp2 bit-hack split across DVE (PSUM->SBUF
                            # move) and the otherwise-idle gpsimd engine
                            # (x*A+B -> round -> int16 bits == bf16 2^x).
                            tmp = schp.tile(
                                [128, 1024], bf, tag="sch",
                                name=f"sch{sb}_{pair}_{c}",
                            )
                            nc.vector.tensor_copy(tmp[:], ps[:])
                            nc.gpsimd.tensor_scalar(
                                Pt[:, c * 1024 : (c + 1) * 1024].bitcast(i16),
                                tmp[:], SCH_A, SCH_B,
                                mybir.AluOpType.mult, mybir.AluOpType.add,
                            )
                        else:
                            nc.scalar.activation(
                                Pt[:, c * 1024 : (c + 1) * 1024],
                                ps[:], Exp, scale=SCALE,
                            )
                        if c % 4 == 3:
                            # quarter-granularity mask application: smaller
                            # DVE ops, earlier attnV enablement; one half
                            # goes to the otherwise-idle gpsimd engine
                            qsl = slice(c - 3, c + 1)
                            for h2 in range(2):
                                eng = (
                                    nc.gpsimd if (c == 3 and h2 == 0)
                                    else nc.vector
                                )
                                eng.tensor_mul(
                                    pv[:, qsl, h2, :], pv[:, qsl, h2, :],
                                    mv[:, qsl, :],
                                )
                        fast_drain = prev is not None and prev[:2] != (0, 0)
                        if prev is not None:
                            # drain prev iteration's attnV 2-per-slot so its
                            # norm + Wo can start at mid-iteration; the very
                            # first prev=(0,0) must go 1-per-slot because
                            # vproj(2)/(3) only land mid-(0,1)
                            psb, ppair, pPt, _ = prev
                            ks = (
                                (2 * c, 2 * c + 1) if (fast_drain and c < 8)
                                else (c,) if not fast_drain
                                else ()
                            )
                            for k in ks:
                                for h2 in range(2):
                                    h = ppair * 2 + h2
                                    nc.tensor.matmul(
                                        po2[h2][0:65, :],
                                        vpc[k][:, h * 65 : h * 65 + 65],
                                        pPt[:, (2 * k + h2) * 512 : (2 * k + h2 + 1) * 512],
                                        start=(k == 0), stop=(k == NC_T - 1),
                                    )
                        if last_it and c >= 4:
                            # attnV of the last pair, k = c-4: chunk k is
                            # only read after its quarter-mask (slot
                            # 4*(k//4)+3 <= c) — k=12..15 go in the tail
                            if c == 4:
                                po2L = [
                                    bigp.tile([128, 512], f32, tag="big",
                                              name=f"avL_{h2}")
                                    for h2 in range(2)
                                ]
                            k = c - 4
                            for h2 in range(2):
                                h = pair * 2 + h2
                                nc.tensor.matmul(
                                    po2L[h2][0:65, :],
                                    vpc[k][:, h * 65 : h * 65 + 65],
                                    Pt[:, (2 * k + h2) * 512 : (2 * k + h2 + 1) * 512],
                                    start=(k == 0), stop=False,
                                )
                        if prev is not None and (
                            (fast_drain and c == 8)
                            or (not fast_drain and c == NC_T - 1)
                        ):
                            emit_norm(prev[0], prev[1], po2)
                            if prev[1] == 1:
                                for st in range(4):
                                    for mt in range(2):
                                        extras.append(
                                            lambda s=prev[0], a=st, b=mt:
                                            emit_wo_group(s, a, b)
                                        )
                        if c == 1 and pair == 0 and sb + 1 < NSB:
                            emit_mask_dma(sb + 1, 0)
                            emit_mask_dma(sb + 1, 1)
                            emit_qdma(sb + 1)
                        elif c in (1, 3) and pair == 1 and sb + 1 < NSB:
                            emit_qproj_jt(sb + 1, c // 2)
                        elif extras and (
                            (prev is None and c >= 2)
                            or (c % 2 == 1 and (c >= 5 or (pair == 0 and c >= 3)))
                        ):
                            extras.pop(0)()
                    prev = (sb, pair, Pt, None)
            # tail: last quarter of attnV(3,1), its norm, then Wo
            psb, ppair, pPt, _ = prev
            for k in range(12, NC_T):
                for h2 in range(2):
                    h = ppair * 2 + h2
                    nc.tensor.matmul(
                        po2L[h2][0:65, :],
                        vpc[k][:, h * 65 : h * 65 + 65],
                        pPt[:, (2 * k + h2) * 512 : (2 * k + h2 + 1) * 512],
                        start=False, stop=(k == NC_T - 1),
                    )
            emit_norm(psb, ppair, po2L)
            for fn in extras:
                fn()
            for st in range(4):
                for mt in range(2):
                    emit_wo_group(NSB - 1, st, mt)


def build_nc():
    nc = bacc.Bacc("TRN2", target_bir_lowering=False, debug=False)
    names = {}
    def din(name, shape, dt):
        names[name] = nc.dram_tensor(name, shape, dt, kind="ExternalInput").ap()
    din("qT", [D, S], f8)
    din("kT", [D, S], f8)
    din("vT", [D, S], f8)
    din("maskT", [S, S], bf)
    din("wqT", [D, JC], f8)
    din("wkT", [D, JC], f8)
    din("wvT", [D, JC], f8)
    din("woT", [JC, D], bf)
    # (f8 aliases bf16 now; kept symbolic for easy re-experiments)
    din("biasqk", [128, 4], f32)
    names["out_p"] = nc.dram_tensor(
        "out_p", [S, D], bf, kind="ExternalOutput"
    ).ap()
    with tile_mod.TileContext(nc) as tc:
        _emit(tc, names)
    nc.compile()
    return nc


_NC = None


def prep_inputs(q, k, v, mask, Wq, bq, Wk, bk, Wv, bv, Wo, bo):
    q = np.asarray(q, F32)
    k = np.asarray(k, F32)
    v = np.asarray(v, F32)
    mask = np.asarray(mask)
    Wq, Wk, Wv, Wo = (np.asarray(w, F32) for w in (Wq, Wk, Wv, Wo))
    bq, bk, bv, bo = (np.asarray(b_, F32) for b_ in (bq, bk, bv, bo))

    maskT = np.ascontiguousarray(mask[0, 0].T).astype(BF16)
    qT = [np.ascontiguousarray(q[b_].T).astype(F8) for b_ in range(B)]
    kT = [np.ascontiguousarray(k[b_].T).astype(F8) for b_ in range(B)]
    vT = [np.ascontiguousarray(v[b_].T).astype(F8) for b_ in range(B)]

    in_maps = []
    for c in range(N_CORES):
        b_, g = c // 4, c % 4
        js = slice(g * JC, (g + 1) * JC)
        biasqk = np.stack(
            [bq[js][:128], bq[js][128:], bk[js][:128], bk[js][128:]], axis=1
        ).astype(F32)
        in_maps.append(
            {
                "qT": qT[b_],
                "kT": kT[b_],
                "vT": vT[b_],
                "maskT": maskT,
                "wqT": np.ascontiguousarray(Wq[js, :].T * WPRE).astype(F8),
                "wkT": np.ascontiguousarray(Wk[js, :].T * WPRE).astype(F8),
                "wvT": np.ascontiguousarray(Wv[js, :].T * WPRE).astype(F8),
                "woT": np.ascontiguousarray(Wo[:, js].T).astype(BF16),
                "biasqk": np.ascontiguousarray(biasqk),
            }
        )
    # bv contributes a constant (softmax rows sum to 1): out += Wo @ bv + bo
    bias_out = (Wo @ bv + bo).astype(F32)
    return in_maps, bias_out


def run_prepped(in_maps, bias_out, trace=False, **kw):
    global _NC
    if _NC is None:
        _NC = build_nc()
    res = run_bass_kernel_spmd(
        _NC, in_maps, list(range(N_CORES)), trace=trace, **kw
    )
    out = np.zeros((B, S, D), F32)
    for c in range(N_CORES):
        out[c // 4] += np.asarray(res.results[c]["out_p"], dtype=F32)
    out += bias_out[None, None, :]
    return out, res


def kernel(q, k, v, mask, Wq, bq, Wk, bk, Wv, bv, Wo, bo):
    in_maps, bias_out = prep_inputs(
        q, k, v, mask, Wq, bq, Wk, bk, Wv, bv, Wo, bo
    )
    out, _ = run_prepped(in_maps, bias_out)
    return out



# revision 67
# speedup vs baseline: 1.2505x; 1.2505x over previous
"""Trainium2 Bass kernel for nn_MultiHeadAttention (B=2, S=2048, D=1024, H=16).

Sharding: 8 cores = 2 (batch) x 4 (head groups of 4 heads / 256 dims).
Each core computes QKV projections for its head slice, attention for its 4
heads, and the partial output projection for its 256-dim slice of Wo's input.
Host sums the 4 partials per batch element (Megatron-style row-parallel Wo).

Device layouts (per core):
  qT/kT/vT  [1024, 2048] bf16   (input, transposed on host)
  wqT/wkT/wvT [1024, 256] bf16  (Wq[js].T etc)
  woT       [256, 1024] bf16    (Wo[:, js].T)
  maskT     [2048, 2048] bf16   (mask[0,0].T as 0.0/1.0)
  qpT/kpT   [256(j), 2048(s)]   (projections, transposed: j on partitions)
  vp        [2048(t), 4x65]     (natural layout; col 64 of each 65-block = 1.0
                                 -> attn@V matmul also produces softmax denom)
  P~        [t, s] = exp(scoresT/8) * maskT   (scoresT = K_h.T^T @ Q_h.T)
  attn out  [65(j+denom), s] -> normalized -> concatT [256(j), 2048(s)]
  out_p     [2048, 1024] f32 partial = concatT.T @ woT
"""

import sys

import numpy as np

try:
    import concourse.bass as bass
except ImportError:  # pragma: no cover
    sys.path.insert(0, "/opt/trn_rl_repo")
    import concourse.bass as bass

from concourse import bacc

import ml_dtypes

import concourse.tile as tile_mod
from concourse import mybir
from concourse.bass_utils import run_bass_kernel_spmd

BF16 = ml_dtypes.bfloat16
F32 = np.float32

B, S, D, H = 2, 2048, 1024, 16
DK = D // H            # 64
N_CORES = 8
HPC = 4                # heads per core
JC = HPC * DK          # 256 j-dims per core
SCALE = 1.0 / float(np.sqrt(DK))
NSB = S // 512         # 4 s-blocks
NC_T = S // 128        # 16 t-chunks
VROW = HPC * 65        # 260: [h0 64 | 1 | h1 64 | 1 | ...]

bf = mybir.dt.bfloat16
f32 = mybir.dt.float32
f8 = mybir.dt.bfloat16   # fp8 inputs tried and reverted: logit noise 4.5e-2
F8 = BF16
WPRE = 1.0
WINV = 1.0 / WPRE


def _patch_drain():
    """This walrus build only accepts 1 sync-wait per instruction; the Tile
    exit drain carries one wait per pending proc. Split them across drains."""
    if getattr(tile_mod.TileContext, "_drain_patched", False):
        return
    import bass_rust

    def _drain_and_barrier(self, tick_clock, wait_clock):
        from concourse.tile import ScopedClock

        nc = self.nc
        drain_inst = nc.sync.drain()
        wait_clock.add_sem_waits(
            drain_inst.ins, ScopedClock({None: tick_clock.global_clock})
        )
        si = drain_inst.ins.sync_info
        waits = list(si.on_wait)
        if len(waits) > 1:
            drain_inst.ins.sync_info = bass_rust.SyncInfo(
                on_wait=[waits[0]], on_update=list(si.on_update)
            )
            for w in waits[1:]:
                d2 = nc.sync.drain()
                d2.ins.sync_info = bass_rust.SyncInfo(on_wait=[w], on_update=[])
        nc.all_engine_barrier()
        assert self.sems is not None
        popped = nc._tile_sem_poison_stack.pop()
        assert popped is self._sem_poison
        nc.clear_and_free_semaphores(list(self.sems.allocated().values()))
        nc.all_engine_barrier()

    tile_mod.TileContext._drain_and_barrier = _drain_and_barrier
    tile_mod.TileContext._drain_patched = True


SCH_A = SCALE * float(np.log2(np.e)) * 128.0   # exp2 bit-hack scale
SCH_B = 16256.0 - 7.42                          # 127*128 - sigma
DVE_CHUNKS = (8,)                               # chunks exp'd off the ACT engine


def _emit(tc, T):
    nc = tc.nc
    Exp = mybir.ActivationFunctionType.Exp
    i16 = mybir.dt.int16

    from contextlib import ExitStack

    with ExitStack() as ctx:
        persist = ctx.enter_context(tc.tile_pool(name="persist", bufs=1))

        # ---- weights / persistent tiles ----
        # DMA issue order matters: the DMA engines round-robin across active
        # queues, so anything issued early competes with the k/q blocks the
        # head needs first.  Sync queue: wk, wq, biasqk, kT0, qT0, kT1...
        # The wv/wo/mask/v issues are gated behind kproj(0) completion via a
        # dummy gpsimd read of kpT (emitted later).
        wq = persist.tile([128, 8 * JC], f8, tag="wq")
        wk = persist.tile([128, 8 * JC], f8, tag="wk")
        wv = persist.tile([128, 8 * JC], f8, tag="wv")
        for t, name in ((wk, "wkT"), (wq, "wqT")):
            nc.sync.dma_start(t[:], T[name][:, :])
        biasqk = persist.tile([128, 4], f32, tag="biasqk")
        nc.sync.dma_start(biasqk[:], T["biasqk"][:, :])
        ones1 = persist.tile([1, 512], f32, tag="ones1")
        nc.gpsimd.memset(ones1[:], 1.0)
        wo = [persist.tile([128, D], bf, tag=f"wo{i}", name=f"wo{i}") for i in range(2)]

        # per-sb q/k projection tiles ([j, s] transposed layout)
        qpS = [
            [persist.tile([128, 512], bf, tag=f"qp{j}_{s}", name=f"qp{j}_{s}")
             for s in range(NSB)]
            for j in range(2)
        ]
        kpT = [
            [persist.tile([128, 1024], bf, tag=f"kpT{i}_{th}", name=f"kpT{i}_{th}")
             for th in range(2)]
            for i in range(2)
        ]
        # per-chunk v tiles (natural [t, j] layout + ones cols)
        vpc = [persist.tile([128, VROW], bf, tag=f"vp{c}", name=f"vp{c}")
               for c in range(NC_T)]
        concatT = [persist.tile([128, S], bf, tag=f"concatT{i}", name=f"concatT{i}") for i in range(2)]

        wq_v = wq[:].rearrange("p (c j) -> p c j", c=8)
        wk_v = wk[:].rearrange("p (c j) -> p c j", c=8)
        wv_v = wv[:].rearrange("p (c j) -> p c j", c=8)

        q_stream = ctx.enter_context(tc.tile_pool(name="q_stream", bufs=1))
        qtts = {}

        def emit_qdma(sb):
            sl = slice(sb * 512, (sb + 1) * 512)
            qTt = q_stream.tile([128, 8 * 512], f8, tag="qTt", name=f"qTt{sb}")
            nc.sync.dma_start(qTt[:], T["qT"][sb * 128 : (sb + 1) * 128, :])
            qtts[sb] = qTt[:].rearrange("p (c s) -> p c s", c=8)

        def emit_qproj_jt(sb, jt):
            jsl = slice(jt * 128, (jt + 1) * 128)
            ps = bigp.tile([128, 512], f32, tag="big", name=f"pq{sb}_{jt}")
            for c in range(8):
                nc.tensor.matmul(
                    ps[:], wq_v[:, c, jsl], qtts[sb][:, c, :],
                    start=(c == 0), stop=(c == 7),
                )
            nc.vector.tensor_scalar(
                qpS[jt][sb][:], ps[:], WINV, biasqk[:, jt : jt + 1],
                mybir.AluOpType.mult, mybir.AluOpType.add,
            )

        def emit_qproj(sb):
            emit_qdma(sb)
            emit_qproj_jt(sb, 0)
            emit_qproj_jt(sb, 1)

        # ---- attention + output projection ----
        # Chunk-level software pipeline: per t-chunk the PE stream carries
        # scores(i) for both heads (concurrent row-groups), then attnV(i-1)
        # for both heads, plus occasional "extras" (Wo / q-proj / v-proj /
        # mask prefetch). ACT (exp) is the pacing engine; this keeps it fed
        # every chunk while the PE stays dense enough to hold HAM at 8/8.
        if True:
            vstream = ctx.enter_context(tc.tile_pool(name="vstream", bufs=2))
            schp = ctx.enter_context(tc.tile_pool(name="schp", bufs=2))
            maskp = ctx.enter_context(tc.tile_pool(name="maskp", bufs=2))
            ptp = ctx.enter_context(tc.tile_pool(name="ptp", bufs=2))
            smallp = ctx.enter_context(tc.tile_pool(name="smallp", bufs=1))
            outp = ctx.enter_context(tc.tile_pool(name="outp", bufs=2))
            scp = ctx.enter_context(tc.tile_pool(name="scp", bufs=2, space="PSUM"))
            mtiles = {}

        # ---- k/q projections pipelined into the attention loop ----
        bigp = ctx.enter_context(tc.tile_pool(name="bigp", bufs=4, space="PSUM"))
        with tc.tile_pool(name="kv_stream", bufs=2) as kv_stream:
            ktts = {}

            def emit_kdma(sb):
                sl = slice(sb * 512, (sb + 1) * 512)
                kTt = kv_stream.tile([128, 8 * 512], f8, tag="kTt", name=f"kTt{sb}")
                nc.sync.dma_start(kTt[:], T["kT"][sb * 128 : (sb + 1) * 128, :])
                ktts[sb] = kTt[:].rearrange("p (c s) -> p c s", c=8)

            def emit_kproj_jt(sb, jt):
                jsl = slice(jt * 128, (jt + 1) * 128)
                ps = bigp.tile([128, 512], f32, tag="big", name=f"pk{sb}_{jt}")
                for c in range(8):
                    nc.tensor.matmul(
                        ps[:], wk_v[:, c, jsl], ktts[sb][:, c, :],
                        start=(c == 0), stop=(c == 7),
                    )
                nc.vector.tensor_scalar(
                    kpT[jt][sb // 2][:, (sb % 2) * 512 : (sb % 2 + 1) * 512],
                    ps[:], WINV, biasqk[:, 2 + jt : 3 + jt],
                    mybir.AluOpType.mult, mybir.AluOpType.add,
                )

            # Head: get k0/q0 in flight first, project them, then start
            # attention immediately; k1-k3 projections run as extras inside
            # the (0,0) chunk loop (the PE is otherwise idle there since
            # there is no previous attnV to overlap).
            emit_kdma(0)
            emit_qdma(0)
            emit_kdma(1)
            emit_kproj_jt(0, 0)
            emit_kproj_jt(0, 1)
            emit_qproj_jt(0, 0)
            emit_qproj_jt(0, 1)
            emit_kdma(2)
            emit_kdma(3)
            # Gate the second DMA wave (wv/v/mask/wo) behind kproj(0) so the
            # round-robin DMA engines give the head's k/q blocks full
            # bandwidth first.  Tile schedules by data dependency (not
            # program order), so each gated DMA needs a REAL dep: write a
            # corner of its destination tile from kpT first (WAW ordering).
            def gate(dst_corner):
                nc.gpsimd.tensor_copy(dst_corner, kpT[0][0][0:1, 0:2])

            gate(wv[0:1, 0:2])
            nc.gpsimd.dma_start(wv[:], T["wvT"][:, :])




            def emit_mask_dma(sb, half, gated=False):
                # split per t-half so the piece needed by the c==7 mask-mul
                # lands first
                sl = slice(sb * 512, (sb + 1) * 512)
                if half == 0:
                    mT = maskp.tile(
                        [128, NC_T * 512], bf, tag="mT", name=f"mT{sb}"
                    )
                    mtiles[sb] = mT
                mT = mtiles[sb]
                if gated:
                    gate(mT[0:1, half * 4096 : half * 4096 + 2])
                nc.gpsimd.dma_start(
                    mT[:, half * 4096 : (half + 1) * 4096],
                    T["maskT"][
                        sb * 128 : (sb + 1) * 128,
                        half * 4096 : (half + 1) * 4096,
                    ],
                )

            vtts = {}

            def emit_vdma(tb, gated=False):
                sl = slice(tb * 512, (tb + 1) * 512)
                vTt = vstream.tile([128, 8 * 512], f8, tag="vTt", name=f"vTt{tb}")
                if gated:
                    gate(vTt[0:1, 0:2])
                nc.gpsimd.dma_start(vTt[:], T["vT"][tb * 128 : (tb + 1) * 128, :])
                vtts[tb] = vTt[:].rearrange("p (c t) -> p c t", c=8)

            def emit_vproj_tb(tb):
                vTt_v = vtts[tb]
                if tb + 2 < NSB and tb + 2 not in vtts:
                    emit_vdma(tb + 2)
                for tt in range(4):
                    chunk = tb * 4 + tt
                    ps = bigp.tile([128, 512], f32, tag="big", name=f"pv{chunk}")
                    for c in range(8):
                        nc.tensor.matmul(
                            ps[:, 0:JC],
                            vTt_v[:, c, tt * 128 : (tt + 1) * 128],
                            wv_v[:, c, :],
                            start=(c == 0), stop=(c == 7),
                        )
                    vt = vpc[chunk]
                    nc.gpsimd.memset(
                        vt[:].rearrange("p (h d) -> p h d", d=65)[:, :, 64:65],
                        1.0,
                    )
                    dst = vt[:].rearrange("p (h d) -> p h d", h=HPC)[:, :, 0:DK]
                    src = ps[:, 0:JC].rearrange("p (h d) -> p h d", h=HPC)
                    nc.vector.tensor_scalar_mul(dst, src, WINV)

            def emit_wo_group(sb, st, mt):
                s0 = sb * 512 + st * 128
                msl = slice(mt * 512, (mt + 1) * 512)
                pw = bigp.tile([128, 512], f32, tag="big", name=f"pw{sb}_{st}_{mt}")
                for kc in range(2):
                    nc.tensor.matmul(
                        pw[:],
                        concatT[kc][:, s0 : s0 + 128],
                        wo[kc][:, msl],
                        start=(kc == 0), stop=(kc == 1),
                    )
                ot = outp.tile([128, 512], bf, tag="ot", name=f"ot{sb}_{st}_{mt}")
                if sb == NSB - 1:
                    # tail: ACT is idle once the last exp is done
                    nc.scalar.copy(ot[:], pw[:])
                else:
                    nc.vector.tensor_copy(ot[:], pw[:])
                nc.sync.dma_start(T["out_p"][s0 : s0 + 128, msl], ot[:])

            def emit_norm(sb, pair, po2):
                sl = slice(sb * 512, (sb + 1) * 512)
                rc0 = smallp.tile([1, 1024], f32, tag="rc0", name=f"rc0_{sb}_{pair}")
                for h2 in range(2):
                    nc.vector.tensor_copy(
                        rc0[0:1, h2 * 512 : (h2 + 1) * 512], po2[h2][64:65, :]
                    )
                rc = smallp.tile([1, 1024], f32, tag="rc", name=f"rc{sb}_{pair}")
                nc.vector.reciprocal_approx_fast(rc[:], rc0[:])
                for h2 in range(2):
                    h = pair * 2 + h2
                    psl = slice(h2 * 64, h2 * 64 + 64)
                    rb = smallp.tile([64, 512], f32, tag="rb", name=f"rb{sb}_{h}")
                    nc.gpsimd.partition_broadcast(
                        rb[:], rc[0:1, h2 * 512 : (h2 + 1) * 512], channels=64
                    )
                    nc.vector.tensor_mul(
                        concatT[pair][psl, sl], po2[h2][0:64, :], rb[:]
                    )

            emit_mask_dma(0, 0, gated=True)
            emit_vdma(0, gated=True)
            emit_vdma(1, gated=True)
            emit_mask_dma(0, 1, gated=True)
            for i in range(2):
                gate(wo[i][0:1, 0:2])
                nc.gpsimd.dma_start(wo[i][:], T["woT"][i * 128 : (i + 1) * 128, :])
            extras = [
                (lambda s=sbn, j=jt: emit_kproj_jt(s, j))
                for sbn in (1, 2, 3)
                for jt in range(2)
            ]
            po2L = None
            prev = None        # (sb, pair, Pt, po2)
            for sb in range(NSB):
                for pair in range(2):
                    last_it = (sb == NSB - 1 and pair == 1)
                    if sb == 0:
                        extras.append(lambda t=2 * pair: emit_vproj_tb(t))
                        extras.append(lambda t=2 * pair + 1: emit_vproj_tb(t))

                    Pt = ptp.tile(
                        [128, 2 * NC_T * 512], bf, tag="Pt", name=f"Pt{sb}_{pair}"
                    )
                    pv = Pt[:].rearrange("p (c h s) -> p c h s", c=NC_T, h=2)
                    mv = mtiles[sb][:].rearrange("p (c s) -> p c s", c=NC_T)
                    if prev is not None:
                        po2 = [
                            bigp.tile([128, 512], f32, tag="big",
                                      name=f"av{prev[0]}_{prev[1]}_{h2}")
                            for h2 in range(2)
                        ]
                    for c in range(NC_T):
                        ps = scp.tile(
                            [128, 1024], f32, tag="sc", name=f"sc{sb}_{pair}_{c}"
                        )
                        for h2 in range(2):
                            psl = slice(h2 * 64, h2 * 64 + 64)
                            nc.tensor.matmul(
                                ps[:, h2 * 512 : (h2 + 1) * 512],
                                kpT[pair][c // 8][psl, (c % 8) * 128 : (c % 8 + 1) * 128],
                                qpS[pair][sb][psl, :],
                                start=True, stop=True,
                            )
                        if c in DVE_CHUNKS:
                            # exp2 bit-hack split across DVE (PSUM->SBUF
                            # move) and the otherwise-idle gpsimd engine
                            # (x*A+B -> round -> int16 bits == bf16 2^x).
                            tmp = schp.tile(
                                [128, 1024], bf, tag="sch",
                                name=f"sch{sb}_{pair}_{c}",
                            )
                            nc.vector.tensor_copy(tmp[:], ps[:])
                            nc.gpsimd.tensor_scalar(
                                Pt[:, c * 1024 : (c + 1) * 1024].bitcast(i16),
                                tmp[:], SCH_A, SCH_B,
                                mybir.AluOpType.mult, mybir.AluOpType.add,
                            )
                        else:
                            nc.scalar.activation(
                                Pt[:, c * 1024 : (c + 1) * 1024],
                                ps[:], Exp, scale=SCALE,
                            )
                        if c % 4 == 3:
                            # quarter-granularity mask application: smaller
                            # DVE ops, earlier attnV enablement
                            qsl = slice(c - 3, c + 1)
                            for h2 in range(2):
                                nc.vector.tensor_mul(
                                    pv[:, qsl, h2, :], pv[:, qsl, h2, :],
                                    mv[:, qsl, :],
                                )
                        fast_drain = prev is not None and prev[:2] != (0, 0)
                        if prev is not None:
                            # drain prev iteration's attnV 2-per-slot so its
                            # norm + Wo can start at mid-iteration; the very
                            # first prev=(0,0) must go 1-per-slot because
                            # vproj(2)/(3) only land mid-(0,1)
                            psb, ppair, pPt, _ = prev
                            ks = (
                                (2 * c, 2 * c + 1) if (fast_drain and c < 8)
                                else (c,) if not fast_drain
                                else ()
                            )
                            for k in ks:
                                for h2 in range(2):
                                    h = ppair * 2 + h2
                                    nc.tensor.matmul(
                                        po2[h2][0:65, :],
                                        vpc[k][:, h * 65 : h * 65 + 65],
                                        pPt[:, (2 * k + h2) * 512 : (2 * k + h2 + 1) * 512],
                                        start=(k == 0), stop=(k == NC_T - 1),
                                    )
                        if last_it and c >= 4:
                            # attnV of the last pair, k = c-4: chunk k is
                            # only read after its quarter-mask (slot
                            # 4*(k//4)+3 <= c) — k=12..15 go in the tail
                            if c == 4:
                                po2L = [
                                    bigp.tile([128, 512], f32, tag="big",
                                              name=f"avL_{h2}")
                                    for h2 in range(2)
                                ]
                            k = c - 4
                            for h2 in range(2):
                                h = pair * 2 + h2
                                nc.tensor.matmul(
                                    po2L[h2][0:65, :],
                                    vpc[k][:, h * 65 : h * 65 + 65],
                                    Pt[:, (2 * k + h2) * 512 : (2 * k + h2 + 1) * 512],
                                    start=(k == 0), stop=False,
                                )
                        if prev is not None and (
                            (fast_drain and c == 8)
                            or (not fast_drain and c == NC_T - 1)
                        ):
                            emit_norm(prev[0], prev[1], po2)
                            if prev[1] == 1:
                                for st in range(4):
                                    for mt in range(2):
                                        extras.append(
                                            lambda s=prev[0], a=st, b=mt:
                                            emit_wo_group(s, a, b)
                                        )
                        if c == 1 and pair == 0 and sb + 1 < NSB:
                            emit_mask_dma(sb + 1, 0)
                            emit_mask_dma(sb + 1, 1)
                            emit_qdma(sb + 1)
                        elif c in (1, 3) and pair == 1 and sb + 1 < NSB:
                            emit_qproj_jt(sb + 1, c // 2)
                        elif extras and (
                            (prev is None and c >= 2)
                            or (c % 2 == 1 and (c >= 5 or (pair == 0 and c >= 3)))
                        ):
                            extras.pop(0)()
                    prev = (sb, pair, Pt, None)
            # tail: last quarter of attnV(3,1), its norm, then Wo
            psb, ppair, pPt, _ = prev
            for k in range(12, NC_T):
                for h2 in range(2):
                    h = ppair * 2 + h2
                    nc.tensor.matmul(
                        po2L[h2][0:65, :],
                        vpc[k][:, h * 65 : h * 65 + 65],
                        pPt[:, (2 * k + h2) * 512 : (2 * k + h2 + 1) * 512],
                        start=False, stop=(k == NC_T - 1),
                    )
            emit_norm(psb, ppair, po2L)
            for fn in extras:
                fn()
            for st in range(4):
                for mt in range(2):
                    emit_wo_group(NSB - 1, st, mt)


def build_nc():
    nc = bacc.Bacc("TRN2", target_bir_lowering=False, debug=False)
    names = {}
    def din(name, shape, dt):
        names[name] = nc.dram_tensor(name, shape, dt, kind="ExternalInput").ap()
    # All inputs pre-packed host-side into exact SBUF tile layout so every
    # DMA is a straight contiguous copy (8-16KB lines, minimal descriptors).
    din("qT", [NSB * 128, 8 * 512], f8)
    din("kT", [NSB * 128, 8 * 512], f8)
    din("vT", [NSB * 128, 8 * 512], f8)
    din("maskT", [NSB * 128, NC_T * 512], bf)
    din("wqT", [128, 8 * JC], f8)
    din("wkT", [128, 8 * JC], f8)
    din("wvT", [128, 8 * JC], f8)
    din("woT", [JC, D], bf)
    din("biasqk", [128, 4], f32)
    names["out_p"] = nc.dram_tensor(
        "out_p", [S, D], bf, kind="ExternalOutput"
    ).ap()
    with tile_mod.TileContext(nc) as tc:
        _emit(tc, names)
    nc.compile()
    return nc


_NC = None


def prep_inputs(q, k, v, mask, Wq, bq, Wk, bk, Wv, bv, Wo, bo):
    q = np.asarray(q, F32)
    k = np.asarray(k, F32)
    v = np.asarray(v, F32)
    mask = np.asarray(mask)
    Wq, Wk, Wv, Wo = (np.asarray(w, F32) for w in (Wq, Wk, Wv, Wo))
    bq, bk, bv, bo = (np.asarray(b_, F32) for b_ in (bq, bk, bv, bo))

    def pack_dS(xT, dt):
        # [D, S] -> [NSB*128, 8*512]: [sb*128+p, c*512+s] = xT[c*128+p, sb*512+s]
        x = xT.reshape(8, 128, NSB, 512)
        return np.ascontiguousarray(
            x.transpose(2, 1, 0, 3).reshape(NSB * 128, 8 * 512)
        ).astype(dt)

    def pack_w(wT):
        # [D, JC] -> [128, 8*JC]: [p, c*JC+j] = wT[c*128+p, j]
        w = wT.reshape(8, 128, JC)
        return np.ascontiguousarray(w.transpose(1, 0, 2).reshape(128, 8 * JC))

    mT0 = mask[0, 0].T  # [t, s]
    m = mT0.reshape(NC_T, 128, NSB, 512)
    maskT = np.ascontiguousarray(
        m.transpose(2, 1, 0, 3).reshape(NSB * 128, NC_T * 512)
    ).astype(BF16)
    qT = [pack_dS(q[b_].T, F8) for b_ in range(B)]
    kT = [pack_dS(k[b_].T, F8) for b_ in range(B)]
    vT = [pack_dS(v[b_].T, F8) for b_ in range(B)]

    in_maps = []
    for c in range(N_CORES):
        b_, g = c // 4, c % 4
        js = slice(g * JC, (g + 1) * JC)
        biasqk = np.stack(
            [bq[js][:128], bq[js][128:], bk[js][:128], bk[js][128:]], axis=1
        ).astype(F32)
        in_maps.append(
            {
                "qT": qT[b_],
                "kT": kT[b_],
                "vT": vT[b_],
                "maskT": maskT,
                "wqT": pack_w((Wq[js, :].T * WPRE)).astype(F8),
                "wkT": pack_w((Wk[js, :].T * WPRE)).astype(F8),
                "wvT": pack_w((Wv[js, :].T * WPRE)).astype(F8),
                "woT": np.ascontiguousarray(Wo[:, js].T).astype(BF16),
                "biasqk": np.ascontiguousarray(biasqk),
            }
        )
    # bv contributes a constant (softmax rows sum to 1): out += Wo @ bv + bo
    bias_out = (Wo @ bv + bo).astype(F32)
    return in_maps, bias_out


def run_prepped(in_maps, bias_out, trace=False, **kw):
    global _NC
    if _NC is None:
        _NC = build_nc()
    res = run_bass_kernel_spmd(
        _NC, in_maps, list(range(N_CORES)), trace=trace, **kw
    )
    out = np.zeros((B, S, D), F32)
    for c in range(N_CORES):
        out[c // 4] += np.asarray(res.results[c]["out_p"], dtype=F32)
    out += bias_out[None, None, :]
    return out, res


def kernel(q, k, v, mask, Wq, bq, Wk, bk, Wv, bv, Wo, bo):
    in_maps, bias_out = prep_inputs(
        q, k, v, mask, Wq, bq, Wk, bk, Wv, bv, Wo, bo
    )
    out, _ = run_prepped(in_maps, bias_out)
    return out



# revision 76
# speedup vs baseline: 1.2949x; 1.0355x over previous
"""Trainium2 Bass kernel for nn_MultiHeadAttention (B=2, S=2048, D=1024, H=16).

Sharding: 8 cores = 2 (batch) x 4 (head groups of 4 heads / 256 dims).
Each core computes QKV projections for its head slice, attention for its 4
heads, and the partial output projection for its 256-dim slice of Wo's input.
Host sums the 4 partials per batch element (Megatron-style row-parallel Wo).

Device layouts (per core):
  qT/kT/vT  [1024, 2048] bf16   (input, transposed on host)
  wqT/wkT/wvT [1024, 256] bf16  (Wq[js].T etc)
  woT       [256, 1024] bf16    (Wo[:, js].T)
  maskT     [2048, 2048] bf16   (mask[0,0].T as 0.0/1.0)
  qpT/kpT   [256(j), 2048(s)]   (projections, transposed: j on partitions)
  vp        [2048(t), 4x65]     (natural layout; col 64 of each 65-block = 1.0
                                 -> attn@V matmul also produces softmax denom)
  P~        [t, s] = exp(scoresT/8) * maskT   (scoresT = K_h.T^T @ Q_h.T)
  attn out  [65(j+denom), s] -> normalized -> concatT [256(j), 2048(s)]
  out_p     [2048, 1024] f32 partial = concatT.T @ woT
"""

import sys

import numpy as np

try:
    import concourse.bass as bass
except ImportError:  # pragma: no cover
    sys.path.insert(0, "/opt/trn_rl_repo")
    import concourse.bass as bass

from concourse import bacc

import ml_dtypes

import concourse.tile as tile_mod
from concourse import mybir
from concourse.bass_utils import run_bass_kernel_spmd

BF16 = ml_dtypes.bfloat16
F32 = np.float32

B, S, D, H = 2, 2048, 1024, 16
DK = D // H            # 64
N_CORES = 8
HPC = 4                # heads per core
JC = HPC * DK          # 256 j-dims per core
SCALE = 1.0 / float(np.sqrt(DK))
NSB = S // 512         # 4 s-blocks
NC_T = S // 128        # 16 t-chunks
VROW = HPC * 65        # 260: [h0 64 | 1 | h1 64 | 1 | ...]

bf = mybir.dt.bfloat16
f32 = mybir.dt.float32
f8 = mybir.dt.bfloat16   # fp8 inputs tried and reverted: logit noise 4.5e-2
F8 = BF16
WPRE = 1.0
WINV = 1.0 / WPRE


def _patch_drain():
    """This walrus build only accepts 1 sync-wait per instruction; the Tile
    exit drain carries one wait per pending proc. Split them across drains."""
    if getattr(tile_mod.TileContext, "_drain_patched", False):
        return
    import bass_rust

    def _drain_and_barrier(self, tick_clock, wait_clock):
        from concourse.tile import ScopedClock

        nc = self.nc
        drain_inst = nc.sync.drain()
        wait_clock.add_sem_waits(
            drain_inst.ins, ScopedClock({None: tick_clock.global_clock})
        )
        si = drain_inst.ins.sync_info
        waits = list(si.on_wait)
        if len(waits) > 1:
            drain_inst.ins.sync_info = bass_rust.SyncInfo(
                on_wait=[waits[0]], on_update=list(si.on_update)
            )
            for w in waits[1:]:
                d2 = nc.sync.drain()
                d2.ins.sync_info = bass_rust.SyncInfo(on_wait=[w], on_update=[])
        nc.all_engine_barrier()
        assert self.sems is not None
        popped = nc._tile_sem_poison_stack.pop()
        assert popped is self._sem_poison
        nc.clear_and_free_semaphores(list(self.sems.allocated().values()))
        nc.all_engine_barrier()

    tile_mod.TileContext._drain_and_barrier = _drain_and_barrier
    tile_mod.TileContext._drain_patched = True


SCH_A = SCALE * float(np.log2(np.e)) * 128.0   # exp2 bit-hack scale
SCH_B = 16256.0 - 7.42                          # 127*128 - sigma
DVE_CHUNKS = ()                                 # chunks exp'd off the ACT engine


def _emit(tc, T):
    nc = tc.nc
    Exp = mybir.ActivationFunctionType.Exp
    i16 = mybir.dt.int16

    from contextlib import ExitStack

    with ExitStack() as ctx:
        persist = ctx.enter_context(tc.tile_pool(name="persist", bufs=1))

        # ---- weights / persistent tiles ----
        # DMA issue order matters: the DMA engines round-robin across active
        # queues, so anything issued early competes with the k/q blocks the
        # head needs first.  Sync queue: wk, wq, biasqk, kT0, qT0, kT1...
        # The wv/wo/mask/v issues are gated behind kproj(0) completion via a
        # dummy gpsimd read of kpT (emitted later).
        wq = persist.tile([128, 8 * JC], f8, tag="wq")
        wk = persist.tile([128, 8 * JC], f8, tag="wk")
        wv = persist.tile([128, 8 * JC], f8, tag="wv")
        for t, name in ((wk, "wkT"), (wq, "wqT")):
            nc.sync.dma_start(t[:], T[name][:, :])
        biasqk = persist.tile([128, 4], f32, tag="biasqk")
        nc.sync.dma_start(biasqk[:], T["biasqk"][:, :])
        ones1 = persist.tile([1, 512], f32, tag="ones1")
        nc.gpsimd.memset(ones1[:], 1.0)
        wo = [persist.tile([128, D], bf, tag=f"wo{i}", name=f"wo{i}") for i in range(2)]

        # per-sb q/k projection tiles ([j, s] transposed layout)
        qpS = [
            [persist.tile([128, 512], bf, tag=f"qp{j}_{s}", name=f"qp{j}_{s}")
             for s in range(NSB)]
            for j in range(2)
        ]
        kpT = [
            [persist.tile([128, 1024], bf, tag=f"kpT{i}_{th}", name=f"kpT{i}_{th}")
             for th in range(2)]
            for i in range(2)
        ]
        # per-chunk v tiles (natural [t, j] layout + ones cols)
        vpc = [persist.tile([128, VROW], bf, tag=f"vp{c}", name=f"vp{c}")
               for c in range(NC_T)]
        concatT = [persist.tile([128, S], bf, tag=f"concatT{i}", name=f"concatT{i}") for i in range(2)]

        wq_v = wq[:].rearrange("p (c j) -> p c j", c=8)
        wk_v = wk[:].rearrange("p (c j) -> p c j", c=8)
        wv_v = wv[:].rearrange("p (c j) -> p c j", c=8)

        q_stream = ctx.enter_context(tc.tile_pool(name="q_stream", bufs=1))
        qtts = {}

        def emit_qdma(sb):
            sl = slice(sb * 512, (sb + 1) * 512)
            qTt = q_stream.tile([128, 8 * 512], f8, tag="qTt", name=f"qTt{sb}")
            nc.sync.dma_start(qTt[:], T["qT"][sb * 128 : (sb + 1) * 128, :])
            qtts[sb] = qTt[:].rearrange("p (c s) -> p c s", c=8)

        def emit_qproj_jt(sb, jt):
            jsl = slice(jt * 128, (jt + 1) * 128)
            ps = bigp.tile([128, 512], f32, tag="big", name=f"pq{sb}_{jt}")
            for c in range(8):
                nc.tensor.matmul(
                    ps[:], wq_v[:, c, jsl], qtts[sb][:, c, :],
                    start=(c == 0), stop=(c == 7),
                )
            nc.vector.tensor_scalar(
                qpS[jt][sb][:], ps[:], WINV, biasqk[:, jt : jt + 1],
                mybir.AluOpType.mult, mybir.AluOpType.add,
            )

        def emit_qproj(sb):
            emit_qdma(sb)
            emit_qproj_jt(sb, 0)
            emit_qproj_jt(sb, 1)

        # ---- attention + output projection ----
        # Chunk-level software pipeline: per t-chunk the PE stream carries
        # scores(i) for both heads (concurrent row-groups), then attnV(i-1)
        # for both heads, plus occasional "extras" (Wo / q-proj / v-proj /
        # mask prefetch). ACT (exp) is the pacing engine; this keeps it fed
        # every chunk while the PE stays dense enough to hold HAM at 8/8.
        if True:
            vstream = ctx.enter_context(tc.tile_pool(name="vstream", bufs=2))
            schp = ctx.enter_context(tc.tile_pool(name="schp", bufs=2))
            maskp = ctx.enter_context(tc.tile_pool(name="maskp", bufs=2))
            ptp = ctx.enter_context(tc.tile_pool(name="ptp", bufs=2))
            smallp = ctx.enter_context(tc.tile_pool(name="smallp", bufs=1))
            outp = ctx.enter_context(tc.tile_pool(name="outp", bufs=2))
            scp = ctx.enter_context(tc.tile_pool(name="scp", bufs=2, space="PSUM"))
            mtiles = {}

        # ---- k/q projections pipelined into the attention loop ----
        bigp = ctx.enter_context(tc.tile_pool(name="bigp", bufs=4, space="PSUM"))
        with tc.tile_pool(name="kv_stream", bufs=2) as kv_stream:
            ktts = {}

            def emit_kdma(sb):
                sl = slice(sb * 512, (sb + 1) * 512)
                kTt = kv_stream.tile([128, 8 * 512], f8, tag="kTt", name=f"kTt{sb}")
                nc.sync.dma_start(kTt[:], T["kT"][sb * 128 : (sb + 1) * 128, :])
                ktts[sb] = kTt[:].rearrange("p (c s) -> p c s", c=8)

            def emit_kproj_jt(sb, jt):
                jsl = slice(jt * 128, (jt + 1) * 128)
                ps = bigp.tile([128, 512], f32, tag="big", name=f"pk{sb}_{jt}")
                for c in range(8):
                    nc.tensor.matmul(
                        ps[:], wk_v[:, c, jsl], ktts[sb][:, c, :],
                        start=(c == 0), stop=(c == 7),
                    )
                # ACT is idle during the ramp where k-proj runs
                nc.scalar.activation(
                    kpT[jt][sb // 2][:, (sb % 2) * 512 : (sb % 2 + 1) * 512],
                    ps[:], mybir.ActivationFunctionType.Identity,
                    bias=biasqk[:, 2 + jt : 3 + jt], scale=WINV,
                )

            # Head: get k0/q0 in flight first, project them, then start
            # attention immediately; k1-k3 projections run as extras inside
            # the (0,0) chunk loop (the PE is otherwise idle there since
            # there is no previous attnV to overlap).
            emit_kdma(0)
            emit_qdma(0)
            emit_kdma(1)
            emit_kproj_jt(0, 0)
            emit_kproj_jt(0, 1)
            emit_qproj_jt(0, 0)
            emit_qproj_jt(0, 1)
            emit_kdma(2)
            emit_kdma(3)
            # Gate the second DMA wave (wv/v/mask/wo) behind kproj(0) so the
            # round-robin DMA engines give the head's k/q blocks full
            # bandwidth first.  Tile schedules by data dependency (not
            # program order), so each gated DMA needs a REAL dep: write a
            # corner of its destination tile from kpT first (WAW ordering).
            def gate(dst_corner):
                nc.gpsimd.tensor_copy(dst_corner, kpT[0][0][0:1, 0:2])

            gate(wv[0:1, 0:2])
            nc.gpsimd.dma_start(wv[:], T["wvT"][:, :])




            def emit_mask_dma(sb, half, gated=False):
                # split per t-half so the piece needed by the c==7 mask-mul
                # lands first
                sl = slice(sb * 512, (sb + 1) * 512)
                if half == 0:
                    mT = maskp.tile(
                        [128, NC_T * 512], bf, tag="mT", name=f"mT{sb}"
                    )
                    mtiles[sb] = mT
                mT = mtiles[sb]
                if gated:
                    gate(mT[0:1, half * 4096 : half * 4096 + 2])
                nc.gpsimd.dma_start(
                    mT[:, half * 4096 : (half + 1) * 4096],
                    T["maskT"][
                        sb * 128 : (sb + 1) * 128,
                        half * 4096 : (half + 1) * 4096,
                    ],
                )

            vtts = {}

            def emit_vdma(tb, gated=False):
                sl = slice(tb * 512, (tb + 1) * 512)
                vTt = vstream.tile([128, 8 * 512], f8, tag="vTt", name=f"vTt{tb}")
                if gated:
                    gate(vTt[0:1, 0:2])
                nc.gpsimd.dma_start(vTt[:], T["vT"][tb * 128 : (tb + 1) * 128, :])
                vtts[tb] = vTt[:].rearrange("p (c t) -> p c t", c=8)

            def emit_vproj_tb(tb):
                vTt_v = vtts[tb]
                if tb + 2 < NSB and tb + 2 not in vtts:
                    emit_vdma(tb + 2)
                for tt in range(4):
                    chunk = tb * 4 + tt
                    ps = bigp.tile([128, 512], f32, tag="big", name=f"pv{chunk}")
                    for c in range(8):
                        nc.tensor.matmul(
                            ps[:, 0:JC],
                            vTt_v[:, c, tt * 128 : (tt + 1) * 128],
                            wv_v[:, c, :],
                            start=(c == 0), stop=(c == 7),
                        )
                    vt = vpc[chunk]
                    nc.gpsimd.memset(
                        vt[:].rearrange("p (h d) -> p h d", d=65)[:, :, 64:65],
                        1.0,
                    )
                    dst = vt[:].rearrange("p (h d) -> p h d", h=HPC)[:, :, 0:DK]
                    src = ps[:, 0:JC].rearrange("p (h d) -> p h d", h=HPC)
                    # ACT is idle during the ramp where vproj runs
                    nc.scalar.mul(dst, src, WINV)

            def emit_wo_group(sb, st, mt):
                s0 = sb * 512 + st * 128
                msl = slice(mt * 512, (mt + 1) * 512)
                pw = bigp.tile([128, 512], f32, tag="big", name=f"pw{sb}_{st}_{mt}")
                for kc in range(2):
                    nc.tensor.matmul(
                        pw[:],
                        concatT[kc][:, s0 : s0 + 128],
                        wo[kc][:, msl],
                        start=(kc == 0), stop=(kc == 1),
                    )
                ot = outp.tile([128, 512], bf, tag="ot", name=f"ot{sb}_{st}_{mt}")
                if sb == NSB - 1:
                    # tail: ACT is idle once the last exp is done
                    nc.scalar.copy(ot[:], pw[:])
                else:
                    nc.vector.tensor_copy(ot[:], pw[:])
                nc.sync.dma_start(T["out_p"][s0 : s0 + 128, msl], ot[:])

            def emit_norm(sb, pair, po2):
                sl = slice(sb * 512, (sb + 1) * 512)
                rc0 = smallp.tile([1, 1024], f32, tag="rc0", name=f"rc0_{sb}_{pair}")
                for h2 in range(2):
                    nc.vector.tensor_copy(
                        rc0[0:1, h2 * 512 : (h2 + 1) * 512], po2[h2][64:65, :]
                    )
                rc = smallp.tile([1, 1024], f32, tag="rc", name=f"rc{sb}_{pair}")
                nc.vector.reciprocal_approx_fast(rc[:], rc0[:])
                for h2 in range(2):
                    h = pair * 2 + h2
                    psl = slice(h2 * 64, h2 * 64 + 64)
                    rb = smallp.tile([64, 512], f32, tag="rb", name=f"rb{sb}_{h}")
                    nc.gpsimd.partition_broadcast(
                        rb[:], rc[0:1, h2 * 512 : (h2 + 1) * 512], channels=64
                    )
                    nc.vector.tensor_mul(
                        concatT[pair][psl, sl], po2[h2][0:64, :], rb[:]
                    )

            emit_mask_dma(0, 0, gated=True)
            emit_vdma(0, gated=True)
            emit_vdma(1, gated=True)
            emit_mask_dma(0, 1, gated=True)
            for i in range(2):
                gate(wo[i][0:1, 0:2])
                nc.gpsimd.dma_start(wo[i][:], T["woT"][i * 128 : (i + 1) * 128, :])
            extras = [
                (lambda s=sbn, j=jt: emit_kproj_jt(s, j))
                for sbn in (1, 2, 3)
                for jt in range(2)
            ]
            po2L = None
            prev = None        # (sb, pair, Pt, po2)
            for sb in range(NSB):
                for pair in range(2):
                    last_it = (sb == NSB - 1 and pair == 1)
                    if sb == 0:
                        extras.append(lambda t=2 * pair: emit_vproj_tb(t))
                        extras.append(lambda t=2 * pair + 1: emit_vproj_tb(t))

                    Pt = ptp.tile(
                        [128, 2 * NC_T * 512], bf, tag="Pt", name=f"Pt{sb}_{pair}"
                    )
                    pv = Pt[:].rearrange("p (c h s) -> p c h s", c=NC_T, h=2)
                    mv = mtiles[sb][:].rearrange("p (c s) -> p c s", c=NC_T)
                    if prev is not None:
                        po2 = [
                            bigp.tile([128, 512], f32, tag="big",
                                      name=f"av{prev[0]}_{prev[1]}_{h2}")
                            for h2 in range(2)
                        ]
                    for c in range(NC_T):
                        ps = scp.tile(
                            [128, 1024], f32, tag="sc", name=f"sc{sb}_{pair}_{c}"
                        )
                        for h2 in range(2):
                            psl = slice(h2 * 64, h2 * 64 + 64)
                            nc.tensor.matmul(
                                ps[:, h2 * 512 : (h2 + 1) * 512],
                                kpT[pair][c // 8][psl, (c % 8) * 128 : (c % 8 + 1) * 128],
                                qpS[pair][sb][psl, :],
                                start=True, stop=True,
                            )
                        if c in DVE_CHUNKS:
                            # exp2 bit-hack split across DVE (PSUM->SBUF
                            # move) and the otherwise-idle gpsimd engine
                            # (x*A+B -> round -> int16 bits == bf16 2^x).
                            tmp = schp.tile(
                                [128, 1024], bf, tag="sch",
                                name=f"sch{sb}_{pair}_{c}",
                            )
                            nc.vector.tensor_copy(tmp[:], ps[:])
                            nc.gpsimd.tensor_scalar(
                                Pt[:, c * 1024 : (c + 1) * 1024].bitcast(i16),
                                tmp[:], SCH_A, SCH_B,
                                mybir.AluOpType.mult, mybir.AluOpType.add,
                            )
                        else:
                            nc.scalar.activation(
                                Pt[:, c * 1024 : (c + 1) * 1024],
                                ps[:], Exp, scale=SCALE,
                            )
                        if last_it and c >= 12:
                            # last iteration: per-chunk mask so attnV can
                            # finish in-loop (no serial tail chain)
                            qsl = slice(c, c + 1)
                        elif c % 4 == 3:
                            # quarter-granularity mask application: smaller
                            # DVE ops, earlier attnV enablement
                            qsl = slice(c - 3, c + 1)
                        else:
                            qsl = None
                        if qsl is not None:
                            for h2 in range(2):
                                nc.vector.tensor_mul(
                                    pv[:, qsl, h2, :], pv[:, qsl, h2, :],
                                    mv[:, qsl, :],
                                )
                        fast_drain = prev is not None and prev[:2] != (0, 0)
                        if prev is not None:
                            # drain prev iteration's attnV 2-per-slot so its
                            # norm + Wo can start at mid-iteration; the very
                            # first prev=(0,0) must go 1-per-slot because
                            # vproj(2)/(3) only land mid-(0,1)
                            psb, ppair, pPt, _ = prev
                            ks = (
                                (2 * c, 2 * c + 1) if (fast_drain and c < 8)
                                else (c,) if not fast_drain
                                else ()
                            )
                            for k in ks:
                                for h2 in range(2):
                                    h = ppair * 2 + h2
                                    nc.tensor.matmul(
                                        po2[h2][0:65, :],
                                        vpc[k][:, h * 65 : h * 65 + 65],
                                        pPt[:, (2 * k + h2) * 512 : (2 * k + h2 + 1) * 512],
                                        start=(k == 0), stop=(k == NC_T - 1),
                                    )
                        if last_it and c >= 4:
                            # attnV of the last pair; every chunk k is read
                            # only after its mask (quarters end at slots
                            # 3/7/11, per-chunk from slot 12) so the whole
                            # accumulation finishes in-loop
                            if c == 4:
                                po2L = [
                                    bigp.tile([128, 512], f32, tag="big",
                                              name=f"avL_{h2}")
                                    for h2 in range(2)
                                ]
                            ks = (c - 4,) if c < 12 else (2 * c - 16, 2 * c - 15)
                            for k in ks:
                                for h2 in range(2):
                                    h = pair * 2 + h2
                                    nc.tensor.matmul(
                                        po2L[h2][0:65, :],
                                        vpc[k][:, h * 65 : h * 65 + 65],
                                        Pt[:, (2 * k + h2) * 512 : (2 * k + h2 + 1) * 512],
                                        start=(k == 0), stop=(k == NC_T - 1),
                                    )
                        if prev is not None and (
                            (fast_drain and c == 8)
                            or (not fast_drain and c == NC_T - 1)
                        ):
                            emit_norm(prev[0], prev[1], po2)
                            if prev[1] == 1:
                                for st in range(4):
                                    for mt in range(2):
                                        extras.append(
                                            lambda s=prev[0], a=st, b=mt:
                                            emit_wo_group(s, a, b)
                                        )
                        if c == 0 and pair == 1 and sb + 1 < NSB:
                            # mask prefetch here (not at pair 0) keeps the
                            # DMA-bound ramp free for the v blocks
                            emit_mask_dma(sb + 1, 0)
                            emit_mask_dma(sb + 1, 1)
                        if c == 1 and pair == 0 and sb + 1 < NSB:
                            emit_qdma(sb + 1)
                        elif c in (1, 3) and pair == 1 and sb + 1 < NSB:
                            emit_qproj_jt(sb + 1, c // 2)
                        elif extras and (
                            (prev is None and c >= 2)
                            or (c % 2 == 1 and (
                                c >= 5 or (pair == 0 and c >= 3)
                                or sb == NSB - 1
                            ))
                        ):
                            extras.pop(0)()
                    prev = (sb, pair, Pt, None)
            # tail: norm of the last pair (attnV fully drained in-loop), Wo
            psb, ppair, pPt, _ = prev
            emit_norm(psb, ppair, po2L)
            for fn in extras:
                fn()
            for st in range(4):
                for mt in range(2):
                    emit_wo_group(NSB - 1, st, mt)


def build_nc():
    nc = bacc.Bacc("TRN2", target_bir_lowering=False, debug=False)
    names = {}
    def din(name, shape, dt):
        names[name] = nc.dram_tensor(name, shape, dt, kind="ExternalInput").ap()
    # All inputs pre-packed host-side into exact SBUF tile layout so every
    # DMA is a straight contiguous copy (8-16KB lines, minimal descriptors).
    din("qT", [NSB * 128, 8 * 512], f8)
    din("kT", [NSB * 128, 8 * 512], f8)
    din("vT", [NSB * 128, 8 * 512], f8)
    din("maskT", [NSB * 128, NC_T * 512], bf)
    din("wqT", [128, 8 * JC], f8)
    din("wkT", [128, 8 * JC], f8)
    din("wvT", [128, 8 * JC], f8)
    din("woT", [JC, D], bf)
    din("biasqk", [128, 4], f32)
    names["out_p"] = nc.dram_tensor(
        "out_p", [S, D], bf, kind="ExternalOutput"
    ).ap()
    with tile_mod.TileContext(nc) as tc:
        _emit(tc, names)
    nc.compile()
    return nc


_NC = None


def prep_inputs(q, k, v, mask, Wq, bq, Wk, bk, Wv, bv, Wo, bo):
    q = np.asarray(q, F32)
    k = np.asarray(k, F32)
    v = np.asarray(v, F32)
    mask = np.asarray(mask)
    Wq, Wk, Wv, Wo = (np.asarray(w, F32) for w in (Wq, Wk, Wv, Wo))
    bq, bk, bv, bo = (np.asarray(b_, F32) for b_ in (bq, bk, bv, bo))

    def pack_dS(xT, dt):
        # [D, S] -> [NSB*128, 8*512]: [sb*128+p, c*512+s] = xT[c*128+p, sb*512+s]
        x = xT.reshape(8, 128, NSB, 512)
        return np.ascontiguousarray(
            x.transpose(2, 1, 0, 3).reshape(NSB * 128, 8 * 512)
        ).astype(dt)

    def pack_w(wT):
        # [D, JC] -> [128, 8*JC]: [p, c*JC+j] = wT[c*128+p, j]
        w = wT.reshape(8, 128, JC)
        return np.ascontiguousarray(w.transpose(1, 0, 2).reshape(128, 8 * JC))

    mT0 = mask[0, 0].T  # [t, s]
    m = mT0.reshape(NC_T, 128, NSB, 512)
    maskT = np.ascontiguousarray(
        m.transpose(2, 1, 0, 3).reshape(NSB * 128, NC_T * 512)
    ).astype(BF16)
    qT = [pack_dS(q[b_].T, F8) for b_ in range(B)]
    kT = [pack_dS(k[b_].T, F8) for b_ in range(B)]
    vT = [pack_dS(v[b_].T, F8) for b_ in range(B)]

    in_maps = []
    for c in range(N_CORES):
        b_, g = c // 4, c % 4
        js = slice(g * JC, (g + 1) * JC)
        biasqk = np.stack(
            [bq[js][:128], bq[js][128:], bk[js][:128], bk[js][128:]], axis=1
        ).astype(F32)
        in_maps.append(
            {
                "qT": qT[b_],
                "kT": kT[b_],
                "vT": vT[b_],
                "maskT": maskT,
                "wqT": pack_w((Wq[js, :].T * WPRE)).astype(F8),
                "wkT": pack_w((Wk[js, :].T * WPRE)).astype(F8),
                "wvT": pack_w((Wv[js, :].T * WPRE)).astype(F8),
                "woT": np.ascontiguousarray(Wo[:, js].T).astype(BF16),
                "biasqk": np.ascontiguousarray(biasqk),
            }
        )
    # bv contributes a constant (softmax rows sum to 1): out += Wo @ bv + bo
    bias_out = (Wo @ bv + bo).astype(F32)
    return in_maps, bias_out


def run_prepped(in_maps, bias_out, trace=False, **kw):
    global _NC
    if _NC is None:
        _NC = build_nc()
    res = run_bass_kernel_spmd(
        _NC, in_maps, list(range(N_CORES)), trace=trace, **kw
    )
    out = np.zeros((B, S, D), F32)
    for c in range(N_CORES):
        out[c // 4] += np.asarray(res.results[c]["out_p"], dtype=F32)
    out += bias_out[None, None, :]
    return out, res


def kernel(q, k, v, mask, Wq, bq, Wk, bk, Wv, bv, Wo, bo):
    in_maps, bias_out = prep_inputs(
        q, k, v, mask, Wq, bq, Wk, bk, Wv, bv, Wo, bo
    )
    out, _ = run_prepped(in_maps, bias_out)
    return out

